# revision 23
# baseline (speedup 1.0000x reference)
"""GraphWaveNet encoder on 8 Trainium2 NeuronCores (Bass/Tile), v2.

Structure (graph-level data parallel; 512 graphs -> 8 cores x 64 graphs;
batch_idx sorted so each core owns a contiguous node range, padded to NPAD;
within a core, nodes are reordered by descending degree):

  - Layer 1: the per-edge message stream (dinv[src]*x[src], incl. self rows)
    is pre-gathered ON THE HOST in rank-major staircase order, so the device
    just streams it sequentially and aggregates with identity-rhs matmuls
    (transpose-accumulate into PSUM). No on-device gather, no one-hot builds.
  - BN(eval)+bias folded: W*bnscale on host; bias applied via a K=1 matmul
    with a sqrt(deg) row; dst-side norm via the ReLU activation scale
    (dinv^2 for layer 1 so h1_local stores dinv*h1; dinv for layer 2).
  - h1_local is allgathered (bf16) to a replicated h1_all table.
  - Layer 2: dma_gather of h1_all rows per edge (dst-block-grouped, window-
    split for the int16 index limit, tight r16 caps + zero-row pads), scatter
    via is_equal one-hots (f32 iota, single op, DVE) + PE matmuls, window
    partials accumulated in SBUF.
  - Per-graph mean pool: host-built one-hot (1/cnt) matmul.
  - Edge encoder: 17-row ea (bias ones and 1/ecnt folded on host),
    relu(ea17^T @ [We1;be1]) summed per graph via is_equal one-hots,
    then @We2 + masked be2.  Interleaved with both GCN layers for overlap.
"""

import numpy as np
import ml_dtypes

import concourse.bass as bass
import concourse.bacc as bacc
import concourse.mybir as mybir
import concourse.tile as tile
from concourse.library_config import mlp

N = 100000
E = 1600000
D = 128
DE = 16
G = 512
EPS = 1e-5
NC = 8
GPC = G // NC
P = 128
W = 32768
NPAD = 12800
NB = NPAD // P
NTOT = NC * NPAD
NWIN = (NTOT + W - 1) // W
GRP = 3                      # L2 blocks per gather group
NGRP = (NB + GRP - 1) // GRP

F32 = mybir.dt.float32
BF16 = mybir.dt.bfloat16
I16 = mybir.dt.int16
BF = ml_dtypes.bfloat16

_cache = {}


def _r(x, m):
    return (x + m - 1) // m * m


def _pack_idx(vals):
    """[n] int16 -> [128, n//16] wrapped in 16 partitions, replicated x8."""
    n = vals.shape[0]
    t = vals.reshape(n // 16, 16).T.astype(np.int16)
    return np.tile(t, (8, 1))


def _build_nc(sched):
    (K1, caps2, TE, TOT1, TI2, NOH2, S2G, gspecs, ohspecs, ZW) = sched
    K1MAX = max(K1) if K1 else 1
    S2GMAX = max(S2G)

    nc = bacc.Bacc("TRN2", target_bir_lowering=False, debug=False,
                   num_devices=NC, num_swdge_queues=2)

    xg = nc.dram_tensor("xg", [TOT1, D], BF16, kind="ExternalInput")
    idx2 = nc.dram_tensor("idx2", [P, TI2 // 16], I16, kind="ExternalInput")
    meta2 = nc.dram_tensor("meta2", [P, NOH2], F32, kind="ExternalInput")
    dinvb = nc.dram_tensor("dinvb", [P, NB], F32, kind="ExternalInput")
    dsqb = nc.dram_tensor("dsqb", [P, NB], F32, kind="ExternalInput")
    invd = nc.dram_tensor("invd", [1, NPAD], BF16, kind="ExternalInput")
    pool_oh = nc.dram_tensor("pool_oh", [NB, P, GPC], BF16,
                             kind="ExternalInput")
    ea17 = nc.dram_tensor("ea17", [17, TE * P], BF16, kind="ExternalInput")
    egr = nc.dram_tensor("egr", [P, TE], F32, kind="ExternalInput")
    w1f = nc.dram_tensor("w1f", [D, D], BF16, kind="ExternalInput")
    t1r = nc.dram_tensor("t1r", [1, D], BF16, kind="ExternalInput")
    w2f = nc.dram_tensor("w2f", [D, D], BF16, kind="ExternalInput")
    t2r = nc.dram_tensor("t2r", [1, D], BF16, kind="ExternalInput")
    we1b = nc.dram_tensor("we1b", [17, D], BF16, kind="ExternalInput")
    we2 = nc.dram_tensor("we2", [D, D], BF16, kind="ExternalInput")
    be2c = nc.dram_tensor("be2c", [1, D], BF16, kind="ExternalInput")
    emask = nc.dram_tensor("emask", [1, GPC], BF16, kind="ExternalInput")
    io128 = nc.dram_tensor("io128", [P, P], F32, kind="ExternalInput")
    iog = nc.dram_tensor("iog", [P, GPC], F32, kind="ExternalInput")
    identd = nc.dram_tensor("identd", [P, P], BF16, kind="ExternalInput")

    out_t = nc.dram_tensor("out_t", [D, GPC], F32, kind="ExternalOutput")

    h1_local = nc.dram_tensor("h1_local", [NPAD, D], BF16)
    h1_all = nc.dram_tensor("h1_all", [NTOT, D], BF16, addr_space="Shared")

    EPG = 16                 # encoder tiles per slab
    NEG = TE // EPG          # encoder slabs

    with tile.TileContext(nc) as tc:
        with (
            tc.tile_pool(name="const", bufs=1) as cpool,
            tc.tile_pool(name="sb", bufs=3) as pool,
            tc.tile_pool(name="ohpool", bufs=48) as ohp,
            tc.tile_pool(name="big", bufs=2) as bigp,
            tc.tile_pool(name="g2p", bufs=4) as g2p,
            tc.tile_pool(name="ixp", bufs=6) as ixp,
            tc.tile_pool(name="ps", bufs=2, space="PSUM") as psum,
            tc.tile_pool(name="ps1", bufs=2, space="PSUM") as psum1,
            tc.tile_pool(name="acc", bufs=1, space="PSUM") as psacc,
        ):
            nc.gpsimd.load_library(mlp)

            cio = cpool.tile([P, P], F32)
            nc.sync.dma_start(out=cio[:], in_=io128[:])
            ciog = cpool.tile([P, GPC], F32)
            nc.sync.dma_start(out=ciog[:], in_=iog[:])
            cid = cpool.tile([P, P], BF16)
            nc.sync.dma_start(out=cid[:], in_=identd[:])
            cdinv = cpool.tile([P, NB], F32)
            nc.sync.dma_start(out=cdinv[:], in_=dinvb[:])
            cdsq = cpool.tile([P, NB], F32)
            nc.sync.dma_start(out=cdsq[:], in_=dsqb[:])
            cinvd = cpool.tile([1, NPAD], BF16)
            nc.sync.dma_start(out=cinvd[:], in_=invd[:])
            cmeta2 = cpool.tile([P, NOH2], F32)
            nc.sync.dma_start(out=cmeta2[:], in_=meta2[:])
            cegr = cpool.tile([P, TE], F32)
            nc.sync.dma_start(out=cegr[:], in_=egr[:])
            cw1 = cpool.tile([D, D], BF16)
            nc.sync.dma_start(out=cw1[:], in_=w1f[:])
            ct1 = cpool.tile([1, D], BF16)
            nc.sync.dma_start(out=ct1[:], in_=t1r[:])
            cw2 = cpool.tile([D, D], BF16)
            nc.sync.dma_start(out=cw2[:], in_=w2f[:])
            ct2 = cpool.tile([1, D], BF16)
            nc.sync.dma_start(out=ct2[:], in_=t2r[:])
            cwe1 = cpool.tile([17, D], BF16)
            nc.sync.dma_start(out=cwe1[:], in_=we1b[:])
            cwe2 = cpool.tile([D, D], BF16)
            nc.sync.dma_start(out=cwe2[:], in_=we2[:])
            cbe2 = cpool.tile([1, D], BF16)
            nc.sync.dma_start(out=cbe2[:], in_=be2c[:])
            cmask = cpool.tile([1, GPC], BF16)
            nc.sync.dma_start(out=cmask[:], in_=emask[:])

            pool_ps = psacc.tile([D, GPC], F32, space="PSUM")
            es_ps = psacc.tile([D, GPC], F32, space="PSUM")

            # pre-zero both g2 ring buffers once (overhang slots are
            # excluded by -1 drel one-hots, but must not be NaN garbage)
            for _ in range(4):
                gz = g2p.tile([P, (S2GMAX // P) * D], BF16, tag="g2")
                nc.vector.memset(gz[:], 0.0)

            # ---- edge encoder slab generator (interleaved) ----
            def enc_slab(s):
                t0 = s * EPG
                ea = bigp.tile([17, EPG * P], BF16, tag="ea")
                nc.sync.dma_start(out=ea[:], in_=ea17[:, t0 * P:(t0 + EPG) * P])
                for q in range(EPG // 4):
                    e_ps = psum1.tile([P, 4 * D], F32, space="PSUM", tag="eps")
                    for j in range(4):
                        t = t0 + q * 4 + j
                        nc.tensor.matmul(
                            out=e_ps[:, j * D:(j + 1) * D],
                            lhsT=ea[:, (q * 4 + j) * P:(q * 4 + j + 1) * P],
                            rhs=cwe1[:], start=True, stop=True)
                    he = pool.tile([P, 4 * D], BF16, tag="he")
                    nc.scalar.activation(
                        out=he[:], in_=e_ps[:],
                        func=mybir.ActivationFunctionType.Relu)
                    for j in range(4):
                        t = t0 + q * 4 + j
                        ohe = pool.tile([P, GPC], BF16, tag="ohe")
                        nc.vector.tensor_scalar(
                            out=ohe[:], in0=ciog[:],
                            scalar1=cegr[:, t:t + 1], scalar2=None,
                            op0=mybir.AluOpType.is_equal)
                        nc.tensor.matmul(
                            out=es_ps[:], lhsT=he[:, j * D:(j + 1) * D],
                            rhs=ohe[:], start=(t == 0), stop=(t == TE - 1))

            enc_done = 0

            def enc_issue(n):
                nonlocal enc_done
                for _ in range(n):
                    if enc_done < NEG:
                        enc_slab(enc_done)
                        enc_done += 1

            # ---- layer 1: host-pregathered stream, identity aggregation ----
            off = 0
            for b in range(NB):
                k1 = K1[b]
                h_ps = psum1.tile([P, D], F32, space="PSUM", tag="hps")
                if k1 > 0:
                    slab = bigp.tile([P, K1MAX * D], BF16, tag="slab")
                    nc.sync.dma_start(
                        out=slab[:, :k1 * D].rearrange("p (k d) -> p k d", d=D),
                        in_=xg[off:off + P * k1].rearrange(
                            "(p k) d -> p k d", k=k1))
                    agg_ps = psum.tile([D, P], F32, space="PSUM", tag="agg")
                    for k in range(k1):
                        nc.tensor.matmul(
                            out=agg_ps[:], lhsT=slab[:, k * D:(k + 1) * D],
                            rhs=cid[:], start=(k == 0), stop=(k == k1 - 1))
                    agg1 = pool.tile([D, P], BF16, tag="agg1")
                    nc.scalar.copy(out=agg1[:], in_=agg_ps[:])
                    nc.tensor.matmul(out=h_ps[:], lhsT=agg1[:], rhs=cw1[:],
                                     start=True, stop=False)
                    nc.tensor.matmul(out=h_ps[:],
                                     lhsT=cinvd[:, b * P:(b + 1) * P],
                                     rhs=ct1[:], start=False, stop=True)
                else:
                    nc.tensor.matmul(out=h_ps[:],
                                     lhsT=cinvd[:, b * P:(b + 1) * P],
                                     rhs=ct1[:], start=True, stop=True)
                h1s = pool.tile([P, D], BF16, tag="h1s")
                nc.scalar.activation(
                    out=h1s[:], in_=h_ps[:],
                    func=mybir.ActivationFunctionType.Relu,
                    scale=cdsq[:, b:b + 1])
                nc.sync.dma_start(out=h1_local[b * P:(b + 1) * P, :],
                                  in_=h1s[:])
                off += P * k1
                if b % 2 == 1:
                    enc_issue(1)

            nc.gpsimd.collective_compute(
                "AllGather", mybir.AluOpType.bypass,
                replica_groups=[list(range(NC))],
                ins=[h1_local[:]], outs=[h1_all[:]],
            )

            # ---- layer 2: dma_gather + one-hot scatter ----
            for g in range(NGRP):
                b0 = g * GRP
                nblk = min(GRP, NB - b0)
                s2g = S2G[g]
                if not gspecs[g]:
                    specs_g = []
                else:
                    specs_g = gspecs[g]
                g2 = g2p.tile([P, (S2GMAX // P) * D], BF16, tag="g2")
                if specs_g:
                    ix = ixp.tile([P, S2GMAX // 16], I16, tag="ix")
                    icol0 = specs_g[0][3]
                    nc.sync.dma_start(
                        out=ix[:, :s2g // 16],
                        in_=idx2[:, icol0 // 16:(icol0 + s2g) // 16])
                for qi, (slot0, n_gw, wbase, icol, used) in enumerate(specs_g):
                    wrows = min(W, NTOT - wbase)
                    nc.gpsimd.dma_gather(
                        g2[:, (slot0 // P) * D:((slot0 + n_gw) // P) * D]
                        .rearrange("p (c d) -> p c d", d=D),
                        h1_all[wbase:wbase + wrows],
                        ix[:, (icol - icol0) // 16:
                           (icol - icol0 + n_gw) // 16],
                        num_idxs=n_gw, num_idxs_reg=n_gw,
                        elem_size=D, single_packet=False,
                        queue_num=(g * NWIN + qi) % 2,
                    )
                aggsb = pool.tile([D, GRP * P], BF16, tag="aggsb")
                hs_tiles = {}
                for brel in range(nblk):
                    hs = pool.tile([P, D], BF16, tag="hs", bufs=8)
                    nc.sync.dma_start(
                        out=hs[:],
                        in_=h1_local[(b0 + brel) * P:(b0 + brel + 1) * P, :])
                    hs_tiles[brel] = hs
                for w in range(NWIN):
                    segs = [s for s in ohspecs[g] if s[0] == w]
                    if not segs:
                        continue
                    # pass 1: build this window's one-hots (no data deps,
                    # runs ahead of the gather on DVE)
                    ohtiles = []
                    for (_, brel, first_w, entries) in segs:
                        for (chunk, mcol) in entries:
                            oh = ohp.tile([P, P], BF16, tag="oh")
                            nc.vector.tensor_scalar(
                                out=oh[:], in0=cio[:],
                                scalar1=cmeta2[:, mcol:mcol + 1],
                                scalar2=None,
                                op0=mybir.AluOpType.is_equal)
                            ohtiles.append(oh)
                    # pass 2: scatter matmuls + window accumulation
                    oi = 0
                    for (_, brel, first_w, entries) in segs:
                        agg_ps = psum.tile([D, P], F32, space="PSUM",
                                           tag="agg")
                        if first_w:
                            nc.tensor.matmul(
                                out=agg_ps[:], lhsT=hs_tiles[brel][:],
                                rhs=cid[:], start=True,
                                stop=(len(entries) == 0))
                        for j, (chunk, mcol) in enumerate(entries):
                            nc.tensor.matmul(
                                out=agg_ps[:],
                                lhsT=g2[:, chunk * D:(chunk + 1) * D],
                                rhs=ohtiles[oi][:],
                                start=(j == 0 and not first_w),
                                stop=(j == len(entries) - 1))
                            oi += 1
                        if first_w:
                            nc.vector.tensor_copy(
                                out=aggsb[:, brel * P:(brel + 1) * P],
                                in_=agg_ps[:])
                        else:
                            atmp = pool.tile([D, P], BF16, tag="atmp")
                            nc.vector.tensor_copy(out=atmp[:], in_=agg_ps[:])
                            nc.vector.tensor_tensor(
                                out=aggsb[:, brel * P:(brel + 1) * P],
                                in0=aggsb[:, brel * P:(brel + 1) * P],
                                in1=atmp[:], op=mybir.AluOpType.add)
                covered = set(br for (w, br, fw, en) in ohspecs[g])
                for brel in range(nblk):
                    b = b0 + brel
                    h_ps = psum1.tile([P, D], F32, space="PSUM", tag="hps")
                    if brel in covered:
                        nc.tensor.matmul(
                            out=h_ps[:],
                            lhsT=aggsb[:, brel * P:(brel + 1) * P],
                            rhs=cw2[:], start=True, stop=False)
                        nc.tensor.matmul(
                            out=h_ps[:], lhsT=cinvd[:, b * P:(b + 1) * P],
                            rhs=ct2[:], start=False, stop=True)
                    else:
                        nc.tensor.matmul(
                            out=h_ps[:], lhsT=cinvd[:, b * P:(b + 1) * P],
                            rhs=ct2[:], start=True, stop=True)
                    h2s = pool.tile([P, D], BF16, tag="h2s")
                    nc.scalar.activation(
                        out=h2s[:], in_=h_ps[:],
                        func=mybir.ActivationFunctionType.Relu,
                        scale=cdinv[:, b:b + 1])
                    po = pool.tile([P, GPC], BF16, tag="po")
                    nc.sync.dma_start(out=po[:], in_=pool_oh[b])
                    nc.tensor.matmul(out=pool_ps[:], lhsT=h2s[:], rhs=po[:],
                                     start=(b == 0), stop=(b == NB - 1))
                enc_issue(2)

            enc_issue(NEG)

            # ---- finalize ----
            es_sb = pool.tile([D, GPC], BF16, tag="essb")
            nc.scalar.copy(out=es_sb[:], in_=es_ps[:])
            er_ps = psum1.tile([D, GPC], F32, space="PSUM", tag="hps")
            nc.tensor.matmul(out=er_ps[:], lhsT=cwe2[:], rhs=es_sb[:],
                             start=True, stop=False)
            nc.tensor.matmul(out=er_ps[:], lhsT=cbe2[:], rhs=cmask[:],
                             start=False, stop=True)
            pl = pool.tile([D, GPC], F32, tag="pl")
            nc.vector.tensor_copy(out=pl[:], in_=pool_ps[:])
            fin = pool.tile([D, GPC], F32, tag="fin")
            nc.vector.tensor_tensor(out=fin[:], in0=pl[:], in1=er_ps[:],
                                    op=mybir.AluOpType.add)
            nc.sync.dma_start(out=out_t[:], in_=fin[:])

    nc.compile()
    return nc


def _preprocess(x, edge_index, batch_idx, edge_attr,
                W1, b1, g1, bt1, m1, v1, W2, b2, g2, bt2, m2, v2,
                We1, be1, We2, be2):
    batch = np.asarray(batch_idx).astype(np.int64)
    src = np.asarray(edge_index[0]).astype(np.int64)
    dst = np.asarray(edge_index[1]).astype(np.int64)
    x = np.asarray(x, dtype=np.float32)
    ea = np.asarray(edge_attr, dtype=np.float32)

    node_core = batch // GPC
    core_start = np.searchsorted(batch, np.arange(NC) * GPC)
    counts = np.append(core_start[1:], N) - core_start
    assert counts.max() <= NPAD

    deg = np.bincount(dst, minlength=N).astype(np.int64) + 1  # incl self
    dinv = (1.0 / np.sqrt(deg)).astype(np.float32)

    # per-core degree-descending permutation
    order = np.lexsort((-deg, node_core))
    oc = node_core[order]
    ostart = np.searchsorted(oc, np.arange(NC))
    within = np.arange(N) - ostart[oc]
    pid = np.empty(N, np.int64)
    pid[order] = oc * NPAD + within

    pidl = pid % NPAD
    blk_n = pidl // P
    prel_n = pidl % P

    # ---- L1 schedule ----
    k1need = np.zeros((NC, NB), np.int64)
    np.maximum.at(k1need, (node_core, blk_n), deg)
    K1 = [int(v) for v in k1need.max(axis=0)]
    OFF1 = np.concatenate([[0], np.cumsum(np.array(K1) * P)])
    TOT1 = int(OFF1[-1])

    # ---- L1 stream ----
    xdv = (x * dinv[:, None]).astype(BF)
    xg_all = np.zeros((NC, TOT1, D), BF)
    k1arr = np.array(K1, np.int64)
    rows_self = OFF1[blk_n] + prel_n * k1arr[blk_n]
    xg_all[node_core, rows_self] = xdv

    dorder = np.argsort(pid[dst], kind="stable")
    sd = pid[dst][dorder]
    rank = np.arange(E) - np.searchsorted(sd, sd) + 1
    d_core = sd // NPAD
    d_blk = (sd % NPAD) // P
    d_prel = sd % P
    assert (rank < k1arr[d_blk]).all() or (rank <= deg[dst][dorder] - 1).all()
    rows_e = OFF1[d_blk] + d_prel * k1arr[d_blk] + rank
    xg_all[d_core, rows_e] = xdv[src[dorder]]

    # ---- L2 schedule + data ----
    # items = edges only; the self-loop term (= h1' row itself) is added
    # from h1_local directly with an identity matmul per block
    i_src = pid[src]
    i_dst = pid[dst]
    i_core = i_dst // NPAD
    i_blk = (i_dst % NPAD) // P
    i_w = i_src // W
    i_drel = (i_dst % P).astype(np.float32)

    key = ((i_core * NB + i_blk) * NWIN + i_w)
    assert (np.diff(key) >= 0).all() or True
    korder = np.argsort(key, kind="stable")
    key = key[korder]
    i_src, i_drel = i_src[korder], i_drel[korder]
    bounds = np.searchsorted(key, np.arange(NC * NB * NWIN + 1))
    cnt = (bounds[1:] - bounds[:-1]).reshape(NC, NB, NWIN)
    cap2 = np.maximum(_rv(cnt.max(axis=0), 16), 0)

    # zero rows (padding pids) per window
    ZW = []
    for w in range(NWIN):
        zc = None
        for c in range(NC):
            zp = c * NPAD + NPAD - 1
            if counts[c] < NPAD and w * W <= zp < (w + 1) * W:
                zc = zp
                break
        assert zc is not None, f"no zero row in window {w}"
        ZW.append(zc)

    # group specs
    gspecs = []   # per group: list of (slot0, n_gw, wbase, icol, used)
    ohspecs = []  # per group: list of (brel, first_w, [(chunk, mcol)...])
    S2G = []
    idx_vals = [[] for _ in range(NC)]
    meta_cols = [[] for _ in range(NC)]
    icol = 0
    for g in range(NGRP):
        b0 = g * GRP
        nblk = min(GRP, NB - b0)
        slot0 = 0
        specs = []
        seen_blk = {}
        entries_by_block = {}
        for w in range(NWIN):
            capsum = int(cap2[b0:b0 + nblk, w].sum())
            n_gw = _r(capsum, P)
            if n_gw == 0:
                continue
            # build idx values per core
            for c in range(NC):
                vals = np.full(n_gw, -1, np.int16)
                pos = 0
                for brel in range(nblk):
                    b = b0 + brel
                    cap = int(cap2[b, w])
                    k = ((c * NB + b) * NWIN + w)
                    s0, s1 = bounds[k], bounds[k + 1]
                    nreal = s1 - s0
                    vals[pos:pos + nreal] = (i_src[s0:s1] - w * W).astype(
                        np.int16)
                    vals[pos + nreal:pos + cap] = ZW[w] - w * W
                    pos += cap
                idx_vals[c].append(vals)
            # oh entries for this window segment
            pos = 0
            for brel in range(nblk):
                b = b0 + brel
                cap = int(cap2[b, w])
                if cap == 0:
                    continue
                gs0 = slot0 + pos          # first slot of this (b,w)
                gs1 = slot0 + pos + cap    # end
                c0, c1 = gs0 // P, (gs1 - 1) // P
                for ch in range(c0, c1 + 1):
                    mcol = None  # per-core meta col values filled below
                    lo = max(gs0, ch * P)
                    hi = min(gs1, (ch + 1) * P)
                    # meta column: drel for slots [lo, hi) of chunk ch
                    for c in range(NC):
                        mv = np.full(P, -1.0, np.float32)
                        k = ((c * NB + b) * NWIN + w)
                        s0 = bounds[k]
                        nreal = bounds[k + 1] - s0
                        # real slots for core c in [lo, hi):
                        roff = lo - gs0
                        cnt_here = hi - lo
                        rr = np.arange(roff, roff + cnt_here)
                        sel = rr < nreal
                        mv[(lo % P) + np.nonzero(sel)[0]] = \
                            i_drel[s0 + rr[sel]]
                        meta_cols[c].append(mv)
                    mcol = len(meta_cols[0]) - 1
                    entries_by_block.setdefault(brel, []).append(
                        (ch, mcol, w))
                pos += cap
            specs.append((slot0, n_gw, w * W, icol, capsum))
            icol += n_gw
            slot0 += n_gw
        S2G.append(slot0 if slot0 > 0 else P)
        gspecs.append(specs)
        ohs = []
        first_seen = set()
        for w in range(NWIN):
            for brel in sorted(entries_by_block):
                sub = [(ch, mc) for (ch, mc, ww) in entries_by_block[brel]
                       if ww == w]
                if not sub:
                    continue
                ohs.append((w, brel, brel not in first_seen, sub))
                first_seen.add(brel)
        for brel in range(nblk):
            if brel not in first_seen:
                ohs.append((0, brel, True, []))
        ohspecs.append(ohs)

    TI2 = icol
    NOH2 = len(meta_cols[0])

    idx2_np = np.zeros((NC, P, TI2 // 16), np.int16)
    for c in range(NC):
        allv = np.concatenate(idx_vals[c])
        idx2_np[c] = _pack_idx(allv)
    meta2_np = np.zeros((NC, P, NOH2), np.float32)
    for c in range(NC):
        meta2_np[c] = np.stack(meta_cols[c], axis=1)

    # ---- per-node scale vectors ----
    dinv_pid = np.zeros(NC * NPAD, np.float32)
    dinv_pid[pid] = dinv
    dinvb_np = dinv_pid.reshape(NC, NB, P).transpose(0, 2, 1).copy()
    dsqb_np = (dinvb_np * dinvb_np)
    invd_pid = np.zeros(NC * NPAD, np.float32)
    invd_pid[pid] = np.sqrt(deg).astype(np.float32)
    invd_np = invd_pid.reshape(NC, 1, NPAD).astype(BF)

    # ---- pooling one-hot ----
    gcnt = np.bincount(batch, minlength=G).astype(np.float32)
    pool_np = np.zeros((NC, NB, P, GPC), np.float32)
    gl = batch - node_core * GPC
    pool_np[node_core, blk_n, prel_n, gl] = 1.0 / np.maximum(gcnt[batch], 1.0)
    pool_np = pool_np.astype(BF)

    # ---- edge encoder ----
    egraph = batch[src]
    ecore = egraph // GPC
    ecnt = np.bincount(egraph, minlength=G).astype(np.float32)
    einv = 1.0 / np.maximum(ecnt, 1.0)
    eorder = np.argsort(ecore, kind="stable")
    ecore_s, egr_s = ecore[eorder], egraph[eorder]
    ebounds = np.searchsorted(ecore_s, np.arange(NC + 1))
    TE = _r(int(np.diff(ebounds).max() + P - 1) // P + 1, 16)
    ea17_np = np.zeros((NC, 17, TE * P), np.float32)
    egr_np = np.full((NC, P, TE), -1.0, np.float32)
    for c in range(NC):
        s0, s1 = ebounds[c], ebounds[c + 1]
        cnt_c = s1 - s0
        sel = eorder[s0:s1]
        sc = einv[egr_s[s0:s1]]
        ea17_np[c, :DE, :cnt_c] = (ea[sel] * sc[:, None]).T
        ea17_np[c, DE, :cnt_c] = sc
        fl = np.arange(cnt_c)
        egr_np[c, fl % P, fl // P] = (egr_s[s0:s1] - c * GPC).astype(
            np.float32)
    ea17_np = ea17_np.astype(BF)
    emask_np = (ecnt.reshape(NC, GPC) > 0).astype(BF)[:, None, :]

    # ---- folded weights ----
    def fold(Wm, bm, gm, btm, mm, vm):
        s = (gm / np.sqrt(vm + EPS)).astype(np.float32)
        wf = (np.asarray(Wm, np.float32) * s[None, :]).astype(BF)
        t = (((np.asarray(bm, np.float32) - mm) * s) + btm).astype(BF)
        return wf, t[None, :]

    w1f_np, t1_np = fold(W1, b1, g1, bt1, m1, v1)
    w2f_np, t2_np = fold(W2, b2, g2, bt2, m2, v2)
    we1b_np = np.concatenate(
        [np.asarray(We1, np.float32),
         np.asarray(be1, np.float32)[None, :]], axis=0).astype(BF)

    common = {
        "w1f": w1f_np, "t1r": t1_np, "w2f": w2f_np, "t2r": t2_np,
        "we1b": we1b_np,
        "we2": np.asarray(We2, np.float32).astype(BF),
        "be2c": np.asarray(be2, np.float32)[None, :].astype(BF),
        "io128": np.tile(np.arange(P, dtype=np.float32)[None, :], (P, 1)),
        "iog": np.tile(np.arange(GPC, dtype=np.float32)[None, :], (P, 1)),
        "identd": np.eye(P, dtype=np.float32).astype(BF),
    }
    in_maps = []
    for c in range(NC):
        m = dict(common)
        m["xg"] = xg_all[c]
        m["idx2"] = idx2_np[c]
        m["meta2"] = meta2_np[c]
        m["dinvb"] = dinvb_np[c]
        m["dsqb"] = dsqb_np[c]
        m["invd"] = invd_np[c]
        m["pool_oh"] = pool_np[c]
        m["ea17"] = ea17_np[c]
        m["egr"] = egr_np[c]
        m["emask"] = emask_np[c]
        in_maps.append(m)

    sched = (tuple(K1), tuple(int(v) for v in cap2.flatten()), TE, TOT1,
             TI2, NOH2, tuple(S2G),
             tuple(tuple(s) for s in gspecs),
             tuple(tuple((w, br, fw, tuple(en)) for (w, br, fw, en) in ohs)
                   for ohs in ohspecs),
             tuple(ZW))
    return in_maps, sched


def _rv(a, m):
    return (a + m - 1) // m * m


def kernel(x, edge_index, batch_idx, edge_attr, num_graphs,
           W1, b1, g1, bt1, m1, v1, W2, b2, g2, bt2, m2, v2,
           We1, be1, We2, be2):
    in_maps, sched = _preprocess(x, edge_index, batch_idx, edge_attr,
                                 W1, b1, g1, bt1, m1, v1,
                                 W2, b2, g2, bt2, m2, v2,
                                 We1, be1, We2, be2)
    key = hash(sched)
    if _cache.get("key") != key:
        nc = _build_nc(sched)
        from concourse.bass_utils import run_bass_kernel_spmd
        _cache["key"] = key
        _cache["nc"] = nc
        _cache["run"] = lambda ims, **kw: run_bass_kernel_spmd(
            _cache["nc"], ims, list(range(NC)), **kw)
    res = _cache["run"](in_maps)
    out = np.zeros((G, D), np.float32)
    for c in range(NC):
        out[c * GPC:(c + 1) * GPC, :] = res.results[c]["out_t"].T
    return out


# revision 24
# speedup vs baseline: 1.0570x; 1.0570x over previous
"""GraphWaveNet encoder on 8 Trainium2 NeuronCores (Bass/Tile), v2.

Structure (graph-level data parallel; 512 graphs -> 8 cores x 64 graphs;
batch_idx sorted so each core owns a contiguous node range, padded to NPAD;
within a core, nodes are reordered by descending degree):

  - Layer 1: the per-edge message stream (dinv[src]*x[src], incl. self rows)
    is pre-gathered ON THE HOST in rank-major staircase order, so the device
    just streams it sequentially and aggregates with identity-rhs matmuls
    (transpose-accumulate into PSUM). No on-device gather, no one-hot builds.
  - BN(eval)+bias folded: W*bnscale on host; bias applied via a K=1 matmul
    with a sqrt(deg) row; dst-side norm via the ReLU activation scale
    (dinv^2 for layer 1 so h1_local stores dinv*h1; dinv for layer 2).
  - h1_local is allgathered (bf16) to a replicated h1_all table.
  - Layer 2: dma_gather of h1_all rows per edge (dst-block-grouped, window-
    split for the int16 index limit, tight r16 caps + zero-row pads), scatter
    via is_equal one-hots (f32 iota, single op, DVE) + PE matmuls, window
    partials accumulated in SBUF.
  - Per-graph mean pool: host-built one-hot (1/cnt) matmul.
  - Edge encoder: 17-row ea (bias ones and 1/ecnt folded on host),
    relu(ea17^T @ [We1;be1]) summed per graph via is_equal one-hots,
    then @We2 + masked be2.  Interleaved with both GCN layers for overlap.
"""

import numpy as np
import ml_dtypes

import concourse.bass as bass
import concourse.bacc as bacc
import concourse.mybir as mybir
import concourse.tile as tile
from concourse.library_config import mlp

N = 100000
E = 1600000
D = 128
DE = 16
G = 512
EPS = 1e-5
NC = 8
GPC = G // NC
P = 128
W = 32768
NPAD = 12800
NB = NPAD // P
NTOT = NC * NPAD
NWIN = (NTOT + W - 1) // W
GRP = 3                      # L2 blocks per gather group
NGRP = (NB + GRP - 1) // GRP

F32 = mybir.dt.float32
BF16 = mybir.dt.bfloat16
I16 = mybir.dt.int16
BF = ml_dtypes.bfloat16

_cache = {}


def _r(x, m):
    return (x + m - 1) // m * m


def _pack_idx(vals):
    """[n] int16 -> [128, n//16] wrapped in 16 partitions, replicated x8."""
    n = vals.shape[0]
    t = vals.reshape(n // 16, 16).T.astype(np.int16)
    return np.tile(t, (8, 1))


def _build_nc(sched):
    (K1, caps2, TE, TOT1, TI2, NOH2, S2G, gspecs, ohspecs, ZW) = sched
    K1MAX = max(K1) if K1 else 1
    S2GMAX = max(S2G)

    nc = bacc.Bacc("TRN2", target_bir_lowering=False, debug=False,
                   num_devices=NC, num_swdge_queues=2)

    xg = nc.dram_tensor("xg", [TOT1, D], BF16, kind="ExternalInput")
    idx2 = nc.dram_tensor("idx2", [P, TI2 // 16], I16, kind="ExternalInput")
    meta2 = nc.dram_tensor("meta2", [P, NOH2], F32, kind="ExternalInput")
    dinvb = nc.dram_tensor("dinvb", [P, NB], F32, kind="ExternalInput")
    dsqb = nc.dram_tensor("dsqb", [P, NB], F32, kind="ExternalInput")
    invd = nc.dram_tensor("invd", [1, NPAD], BF16, kind="ExternalInput")
    pool_oh = nc.dram_tensor("pool_oh", [NB, P, GPC], BF16,
                             kind="ExternalInput")
    ea17 = nc.dram_tensor("ea17", [17, TE * P], BF16, kind="ExternalInput")
    egr = nc.dram_tensor("egr", [P, TE], F32, kind="ExternalInput")
    w1f = nc.dram_tensor("w1f", [D, D], BF16, kind="ExternalInput")
    t1r = nc.dram_tensor("t1r", [1, D], BF16, kind="ExternalInput")
    w2f = nc.dram_tensor("w2f", [D, D], BF16, kind="ExternalInput")
    t2r = nc.dram_tensor("t2r", [1, D], BF16, kind="ExternalInput")
    we1b = nc.dram_tensor("we1b", [17, D], BF16, kind="ExternalInput")
    we2 = nc.dram_tensor("we2", [D, D], BF16, kind="ExternalInput")
    be2c = nc.dram_tensor("be2c", [1, D], BF16, kind="ExternalInput")
    emask = nc.dram_tensor("emask", [1, GPC], BF16, kind="ExternalInput")
    io128 = nc.dram_tensor("io128", [P, P], F32, kind="ExternalInput")
    iog = nc.dram_tensor("iog", [P, GPC], F32, kind="ExternalInput")
    identd = nc.dram_tensor("identd", [P, P], BF16, kind="ExternalInput")

    out_t = nc.dram_tensor("out_t", [D, GPC], F32, kind="ExternalOutput")

    h1_local = nc.dram_tensor("h1_local", [NPAD, D], BF16)
    h1_all = nc.dram_tensor("h1_all", [NTOT, D], BF16, addr_space="Shared")

    EPG = 16                 # encoder tiles per slab
    NEG = TE // EPG          # encoder slabs

    with tile.TileContext(nc) as tc:
        with (
            tc.tile_pool(name="const", bufs=1) as cpool,
            tc.tile_pool(name="sb", bufs=3) as pool,
            tc.tile_pool(name="ohpool", bufs=48) as ohp,
            tc.tile_pool(name="big", bufs=2) as bigp,
            tc.tile_pool(name="g2p", bufs=4) as g2p,
            tc.tile_pool(name="ixp", bufs=6) as ixp,
            tc.tile_pool(name="ps", bufs=2, space="PSUM") as psum,
            tc.tile_pool(name="ps1", bufs=2, space="PSUM") as psum1,
            tc.tile_pool(name="acc", bufs=1, space="PSUM") as psacc,
        ):
            nc.gpsimd.load_library(mlp)

            cio = cpool.tile([P, P], F32)
            nc.sync.dma_start(out=cio[:], in_=io128[:])
            ciog = cpool.tile([P, GPC], F32)
            nc.sync.dma_start(out=ciog[:], in_=iog[:])
            cid = cpool.tile([P, P], BF16)
            nc.sync.dma_start(out=cid[:], in_=identd[:])
            cdinv = cpool.tile([P, NB], F32)
            nc.sync.dma_start(out=cdinv[:], in_=dinvb[:])
            cdsq = cpool.tile([P, NB], F32)
            nc.sync.dma_start(out=cdsq[:], in_=dsqb[:])
            cinvd = cpool.tile([1, NPAD], BF16)
            nc.sync.dma_start(out=cinvd[:], in_=invd[:])
            cmeta2 = cpool.tile([P, NOH2], F32)
            nc.sync.dma_start(out=cmeta2[:], in_=meta2[:])
            cegr = cpool.tile([P, TE], F32)
            nc.sync.dma_start(out=cegr[:], in_=egr[:])
            cw1 = cpool.tile([D, D], BF16)
            nc.sync.dma_start(out=cw1[:], in_=w1f[:])
            ct1 = cpool.tile([1, D], BF16)
            nc.sync.dma_start(out=ct1[:], in_=t1r[:])
            cw2 = cpool.tile([D, D], BF16)
            nc.sync.dma_start(out=cw2[:], in_=w2f[:])
            ct2 = cpool.tile([1, D], BF16)
            nc.sync.dma_start(out=ct2[:], in_=t2r[:])
            cwe1 = cpool.tile([17, D], BF16)
            nc.sync.dma_start(out=cwe1[:], in_=we1b[:])
            cwe2 = cpool.tile([D, D], BF16)
            nc.sync.dma_start(out=cwe2[:], in_=we2[:])
            cbe2 = cpool.tile([1, D], BF16)
            nc.sync.dma_start(out=cbe2[:], in_=be2c[:])
            cmask = cpool.tile([1, GPC], BF16)
            nc.sync.dma_start(out=cmask[:], in_=emask[:])

            pool_ps = psacc.tile([D, GPC], F32, space="PSUM")
            es_ps = psacc.tile([D, GPC], F32, space="PSUM")

            # pre-zero both g2 ring buffers once (overhang slots are
            # excluded by -1 drel one-hots, but must not be NaN garbage)
            for _ in range(4):
                gz = g2p.tile([P, (S2GMAX // P) * D], BF16, tag="g2")
                nc.vector.memset(gz[:], 0.0)

            # ---- edge encoder slab generator (interleaved) ----
            def enc_slab(s):
                t0 = s * EPG
                ea = bigp.tile([17, EPG * P], BF16, tag="ea")
                nc.sync.dma_start(out=ea[:], in_=ea17[:, t0 * P:(t0 + EPG) * P])
                for q in range(EPG // 4):
                    e_ps = psum1.tile([P, 4 * D], F32, space="PSUM", tag="eps")
                    for j in range(4):
                        t = t0 + q * 4 + j
                        nc.tensor.matmul(
                            out=e_ps[:, j * D:(j + 1) * D],
                            lhsT=ea[:, (q * 4 + j) * P:(q * 4 + j + 1) * P],
                            rhs=cwe1[:], start=True, stop=True)
                    he = pool.tile([P, 4 * D], BF16, tag="he")
                    nc.scalar.activation(
                        out=he[:], in_=e_ps[:],
                        func=mybir.ActivationFunctionType.Relu)
                    for j in range(4):
                        t = t0 + q * 4 + j
                        ohe = pool.tile([P, GPC], BF16, tag="ohe")
                        nc.vector.tensor_scalar(
                            out=ohe[:], in0=ciog[:],
                            scalar1=cegr[:, t:t + 1], scalar2=None,
                            op0=mybir.AluOpType.is_equal)
                        nc.tensor.matmul(
                            out=es_ps[:], lhsT=he[:, j * D:(j + 1) * D],
                            rhs=ohe[:], start=(t == 0), stop=(t == TE - 1))

            enc_done = 0

            def enc_issue(n):
                nonlocal enc_done
                for _ in range(n):
                    if enc_done < NEG:
                        enc_slab(enc_done)
                        enc_done += 1

            # ---- layer 1: host-pregathered stream, identity aggregation ----
            off = 0
            for b in range(NB):
                k1 = K1[b]
                h_ps = psum1.tile([P, D], F32, space="PSUM", tag="hps")
                if k1 > 0:
                    slab = bigp.tile([P, K1MAX * D], BF16, tag="slab")
                    nc.sync.dma_start(
                        out=slab[:, :k1 * D].rearrange("p (k d) -> p k d", d=D),
                        in_=xg[off:off + P * k1].rearrange(
                            "(p k) d -> p k d", k=k1))
                    agg_ps = psum.tile([D, P], F32, space="PSUM", tag="agg")
                    for k in range(k1):
                        nc.tensor.matmul(
                            out=agg_ps[:], lhsT=slab[:, k * D:(k + 1) * D],
                            rhs=cid[:], start=(k == 0), stop=(k == k1 - 1))
                    agg1 = pool.tile([D, P], BF16, tag="agg1")
                    nc.scalar.copy(out=agg1[:], in_=agg_ps[:])
                    nc.tensor.matmul(out=h_ps[:], lhsT=agg1[:], rhs=cw1[:],
                                     start=True, stop=False)
                    nc.tensor.matmul(out=h_ps[:],
                                     lhsT=cinvd[:, b * P:(b + 1) * P],
                                     rhs=ct1[:], start=False, stop=True)
                else:
                    nc.tensor.matmul(out=h_ps[:],
                                     lhsT=cinvd[:, b * P:(b + 1) * P],
                                     rhs=ct1[:], start=True, stop=True)
                h1s = pool.tile([P, D], BF16, tag="h1s")
                nc.scalar.activation(
                    out=h1s[:], in_=h_ps[:],
                    func=mybir.ActivationFunctionType.Relu,
                    scale=cdsq[:, b:b + 1])
                nc.sync.dma_start(out=h1_local[b * P:(b + 1) * P, :],
                                  in_=h1s[:])
                off += P * k1
                if b % 2 == 1:
                    enc_issue(1)

            nc.gpsimd.collective_compute(
                "AllGather", mybir.AluOpType.bypass,
                replica_groups=[list(range(NC))],
                ins=[h1_local[:]], outs=[h1_all[:]],
            )

            # ---- layer 2: dma_gather + one-hot scatter ----
            for g in range(NGRP):
                b0 = g * GRP
                nblk = min(GRP, NB - b0)
                s2g = S2G[g]
                if not gspecs[g]:
                    specs_g = []
                else:
                    specs_g = gspecs[g]
                g2 = g2p.tile([P, (S2GMAX // P) * D], BF16, tag="g2")
                if specs_g:
                    ix = ixp.tile([P, S2GMAX // 16], I16, tag="ix")
                    icol0 = specs_g[0][3]
                    nc.sync.dma_start(
                        out=ix[:, :s2g // 16],
                        in_=idx2[:, icol0 // 16:(icol0 + s2g) // 16])
                for qi, (slot0, n_gw, wbase, icol, used) in enumerate(specs_g):
                    wrows = min(W, NTOT - wbase)
                    nc.gpsimd.dma_gather(
                        g2[:, (slot0 // P) * D:((slot0 + n_gw) // P) * D]
                        .rearrange("p (c d) -> p c d", d=D),
                        h1_all[wbase:wbase + wrows],
                        ix[:, (icol - icol0) // 16:
                           (icol - icol0 + n_gw) // 16],
                        num_idxs=n_gw, num_idxs_reg=n_gw,
                        elem_size=D, single_packet=False,
                        queue_num=(g * NWIN + qi) % 2,
                    )
                aggsb = pool.tile([D, GRP * P], BF16, tag="aggsb")
                for w in range(NWIN):
                    segs = [s for s in ohspecs[g] if s[0] == w]
                    if not segs:
                        continue
                    # pass 1: build this window's one-hots (no data deps,
                    # runs ahead of the gather on DVE)
                    ohtiles = []
                    for (_, brel, first_w, entries) in segs:
                        for (chunk, mcol) in entries:
                            oh = ohp.tile([P, P], BF16, tag="oh")
                            nc.vector.tensor_scalar(
                                out=oh[:], in0=cio[:],
                                scalar1=cmeta2[:, mcol:mcol + 1],
                                scalar2=None,
                                op0=mybir.AluOpType.is_equal)
                            ohtiles.append(oh)
                    # pass 2: scatter matmuls + window accumulation
                    oi = 0
                    for (_, brel, first_w, entries) in segs:
                        agg_ps = psum.tile([D, P], F32, space="PSUM",
                                           tag="agg")
                        for j, (chunk, mcol) in enumerate(entries):
                            nc.tensor.matmul(
                                out=agg_ps[:],
                                lhsT=g2[:, chunk * D:(chunk + 1) * D],
                                rhs=ohtiles[oi][:], start=(j == 0),
                                stop=(j == len(entries) - 1))
                            oi += 1
                        if first_w:
                            nc.vector.tensor_copy(
                                out=aggsb[:, brel * P:(brel + 1) * P],
                                in_=agg_ps[:])
                        else:
                            atmp = pool.tile([D, P], BF16, tag="atmp")
                            nc.vector.tensor_copy(out=atmp[:], in_=agg_ps[:])
                            nc.vector.tensor_tensor(
                                out=aggsb[:, brel * P:(brel + 1) * P],
                                in0=aggsb[:, brel * P:(brel + 1) * P],
                                in1=atmp[:], op=mybir.AluOpType.add)
                covered = set(br for (w, br, fw, en) in ohspecs[g])
                for brel in range(nblk):
                    b = b0 + brel
                    h_ps = psum1.tile([P, D], F32, space="PSUM", tag="hps")
                    if brel in covered:
                        nc.tensor.matmul(
                            out=h_ps[:],
                            lhsT=aggsb[:, brel * P:(brel + 1) * P],
                            rhs=cw2[:], start=True, stop=False)
                        nc.tensor.matmul(
                            out=h_ps[:], lhsT=cinvd[:, b * P:(b + 1) * P],
                            rhs=ct2[:], start=False, stop=True)
                    else:
                        nc.tensor.matmul(
                            out=h_ps[:], lhsT=cinvd[:, b * P:(b + 1) * P],
                            rhs=ct2[:], start=True, stop=True)
                    h2s = pool.tile([P, D], BF16, tag="h2s")
                    nc.scalar.activation(
                        out=h2s[:], in_=h_ps[:],
                        func=mybir.ActivationFunctionType.Relu,
                        scale=cdinv[:, b:b + 1])
                    po = pool.tile([P, GPC], BF16, tag="po")
                    nc.sync.dma_start(out=po[:], in_=pool_oh[b])
                    nc.tensor.matmul(out=pool_ps[:], lhsT=h2s[:], rhs=po[:],
                                     start=(b == 0), stop=(b == NB - 1))
                enc_issue(2)

            enc_issue(NEG)

            # ---- finalize ----
            es_sb = pool.tile([D, GPC], BF16, tag="essb")
            nc.scalar.copy(out=es_sb[:], in_=es_ps[:])
            er_ps = psum1.tile([D, GPC], F32, space="PSUM", tag="hps")
            nc.tensor.matmul(out=er_ps[:], lhsT=cwe2[:], rhs=es_sb[:],
                             start=True, stop=False)
            nc.tensor.matmul(out=er_ps[:], lhsT=cbe2[:], rhs=cmask[:],
                             start=False, stop=True)
            pl = pool.tile([D, GPC], F32, tag="pl")
            nc.vector.tensor_copy(out=pl[:], in_=pool_ps[:])
            fin = pool.tile([D, GPC], F32, tag="fin")
            nc.vector.tensor_tensor(out=fin[:], in0=pl[:], in1=er_ps[:],
                                    op=mybir.AluOpType.add)
            nc.sync.dma_start(out=out_t[:], in_=fin[:])

    nc.compile()
    return nc


def _preprocess(x, edge_index, batch_idx, edge_attr,
                W1, b1, g1, bt1, m1, v1, W2, b2, g2, bt2, m2, v2,
                We1, be1, We2, be2):
    batch = np.asarray(batch_idx).astype(np.int64)
    src = np.asarray(edge_index[0]).astype(np.int64)
    dst = np.asarray(edge_index[1]).astype(np.int64)
    x = np.asarray(x, dtype=np.float32)
    ea = np.asarray(edge_attr, dtype=np.float32)

    node_core = batch // GPC
    core_start = np.searchsorted(batch, np.arange(NC) * GPC)
    counts = np.append(core_start[1:], N) - core_start
    assert counts.max() <= NPAD

    deg = np.bincount(dst, minlength=N).astype(np.int64) + 1  # incl self
    dinv = (1.0 / np.sqrt(deg)).astype(np.float32)

    # per-core degree-descending permutation
    order = np.lexsort((-deg, node_core))
    oc = node_core[order]
    ostart = np.searchsorted(oc, np.arange(NC))
    within = np.arange(N) - ostart[oc]
    pid = np.empty(N, np.int64)
    pid[order] = oc * NPAD + within

    pidl = pid % NPAD
    blk_n = pidl // P
    prel_n = pidl % P

    # ---- L1 schedule ----
    k1need = np.zeros((NC, NB), np.int64)
    np.maximum.at(k1need, (node_core, blk_n), deg)
    K1 = [int(v) for v in k1need.max(axis=0)]
    OFF1 = np.concatenate([[0], np.cumsum(np.array(K1) * P)])
    TOT1 = int(OFF1[-1])

    # ---- L1 stream ----
    xdv = (x * dinv[:, None]).astype(BF)
    xg_all = np.zeros((NC, TOT1, D), BF)
    k1arr = np.array(K1, np.int64)
    rows_self = OFF1[blk_n] + prel_n * k1arr[blk_n]
    xg_all[node_core, rows_self] = xdv

    dorder = np.argsort(pid[dst], kind="stable")
    sd = pid[dst][dorder]
    rank = np.arange(E) - np.searchsorted(sd, sd) + 1
    d_core = sd // NPAD
    d_blk = (sd % NPAD) // P
    d_prel = sd % P
    assert (rank < k1arr[d_blk]).all() or (rank <= deg[dst][dorder] - 1).all()
    rows_e = OFF1[d_blk] + d_prel * k1arr[d_blk] + rank
    xg_all[d_core, rows_e] = xdv[src[dorder]]

    # ---- L2 schedule + data ----
    # items = edges + self loops, keyed by dst
    i_src = np.concatenate([pid[src[dorder]], pid])       # source pids
    i_dst = np.concatenate([sd, pid])
    iorder = np.lexsort((i_src // W, i_dst))
    i_src, i_dst = i_src[iorder], i_dst[iorder]
    i_core = i_dst // NPAD
    i_blk = (i_dst % NPAD) // P
    i_w = i_src // W
    i_drel = (i_dst % P).astype(np.float32)

    key = ((i_core * NB + i_blk) * NWIN + i_w)
    assert (np.diff(key) >= 0).all() or True
    korder = np.argsort(key, kind="stable")
    key = key[korder]
    i_src, i_drel = i_src[korder], i_drel[korder]
    bounds = np.searchsorted(key, np.arange(NC * NB * NWIN + 1))
    cnt = (bounds[1:] - bounds[:-1]).reshape(NC, NB, NWIN)
    cap2 = np.maximum(_rv(cnt.max(axis=0), 16), 0)

    # zero rows (padding pids) per window
    ZW = []
    for w in range(NWIN):
        zc = None
        for c in range(NC):
            zp = c * NPAD + NPAD - 1
            if counts[c] < NPAD and w * W <= zp < (w + 1) * W:
                zc = zp
                break
        assert zc is not None, f"no zero row in window {w}"
        ZW.append(zc)

    # group specs
    gspecs = []   # per group: list of (slot0, n_gw, wbase, icol, used)
    ohspecs = []  # per group: list of (brel, first_w, [(chunk, mcol)...])
    S2G = []
    idx_vals = [[] for _ in range(NC)]
    meta_cols = [[] for _ in range(NC)]
    icol = 0
    for g in range(NGRP):
        b0 = g * GRP
        nblk = min(GRP, NB - b0)
        slot0 = 0
        specs = []
        seen_blk = {}
        entries_by_block = {}
        for w in range(NWIN):
            capsum = int(cap2[b0:b0 + nblk, w].sum())
            n_gw = _r(capsum, P)
            if n_gw == 0:
                continue
            # build idx values per core
            for c in range(NC):
                vals = np.full(n_gw, -1, np.int16)
                pos = 0
                for brel in range(nblk):
                    b = b0 + brel
                    cap = int(cap2[b, w])
                    k = ((c * NB + b) * NWIN + w)
                    s0, s1 = bounds[k], bounds[k + 1]
                    nreal = s1 - s0
                    vals[pos:pos + nreal] = (i_src[s0:s1] - w * W).astype(
                        np.int16)
                    vals[pos + nreal:pos + cap] = ZW[w] - w * W
                    pos += cap
                idx_vals[c].append(vals)
            # oh entries for this window segment
            pos = 0
            for brel in range(nblk):
                b = b0 + brel
                cap = int(cap2[b, w])
                if cap == 0:
                    continue
                gs0 = slot0 + pos          # first slot of this (b,w)
                gs1 = slot0 + pos + cap    # end
                c0, c1 = gs0 // P, (gs1 - 1) // P
                for ch in range(c0, c1 + 1):
                    mcol = None  # per-core meta col values filled below
                    lo = max(gs0, ch * P)
                    hi = min(gs1, (ch + 1) * P)
                    # meta column: drel for slots [lo, hi) of chunk ch
                    for c in range(NC):
                        mv = np.full(P, -1.0, np.float32)
                        k = ((c * NB + b) * NWIN + w)
                        s0 = bounds[k]
                        nreal = bounds[k + 1] - s0
                        # real slots for core c in [lo, hi):
                        roff = lo - gs0
                        cnt_here = hi - lo
                        rr = np.arange(roff, roff + cnt_here)
                        sel = rr < nreal
                        mv[(lo % P) + np.nonzero(sel)[0]] = \
                            i_drel[s0 + rr[sel]]
                        meta_cols[c].append(mv)
                    mcol = len(meta_cols[0]) - 1
                    entries_by_block.setdefault(brel, []).append(
                        (ch, mcol, w))
                pos += cap
            specs.append((slot0, n_gw, w * W, icol, capsum))
            icol += n_gw
            slot0 += n_gw
        S2G.append(slot0 if slot0 > 0 else P)
        gspecs.append(specs)
        ohs = []
        first_seen = set()
        for w in range(NWIN):
            for brel in sorted(entries_by_block):
                sub = [(ch, mc) for (ch, mc, ww) in entries_by_block[brel]
                       if ww == w]
                if not sub:
                    continue
                ohs.append((w, brel, brel not in first_seen, sub))
                first_seen.add(brel)
        ohspecs.append(ohs)

    TI2 = icol
    NOH2 = len(meta_cols[0])

    idx2_np = np.zeros((NC, P, TI2 // 16), np.int16)
    for c in range(NC):
        allv = np.concatenate(idx_vals[c])
        idx2_np[c] = _pack_idx(allv)
    meta2_np = np.zeros((NC, P, NOH2), np.float32)
    for c in range(NC):
        meta2_np[c] = np.stack(meta_cols[c], axis=1)

    # ---- per-node scale vectors ----
    dinv_pid = np.zeros(NC * NPAD, np.float32)
    dinv_pid[pid] = dinv
    dinvb_np = dinv_pid.reshape(NC, NB, P).transpose(0, 2, 1).copy()
    dsqb_np = (dinvb_np * dinvb_np)
    invd_pid = np.zeros(NC * NPAD, np.float32)
    invd_pid[pid] = np.sqrt(deg).astype(np.float32)
    invd_np = invd_pid.reshape(NC, 1, NPAD).astype(BF)

    # ---- pooling one-hot ----
    gcnt = np.bincount(batch, minlength=G).astype(np.float32)
    pool_np = np.zeros((NC, NB, P, GPC), np.float32)
    gl = batch - node_core * GPC
    pool_np[node_core, blk_n, prel_n, gl] = 1.0 / np.maximum(gcnt[batch], 1.0)
    pool_np = pool_np.astype(BF)

    # ---- edge encoder ----
    egraph = batch[src]
    ecore = egraph // GPC
    ecnt = np.bincount(egraph, minlength=G).astype(np.float32)
    einv = 1.0 / np.maximum(ecnt, 1.0)
    eorder = np.argsort(ecore, kind="stable")
    ecore_s, egr_s = ecore[eorder], egraph[eorder]
    ebounds = np.searchsorted(ecore_s, np.arange(NC + 1))
    TE = _r(int(np.diff(ebounds).max() + P - 1) // P + 1, 16)
    ea17_np = np.zeros((NC, 17, TE * P), np.float32)
    egr_np = np.full((NC, P, TE), -1.0, np.float32)
    for c in range(NC):
        s0, s1 = ebounds[c], ebounds[c + 1]
        cnt_c = s1 - s0
        sel = eorder[s0:s1]
        sc = einv[egr_s[s0:s1]]
        ea17_np[c, :DE, :cnt_c] = (ea[sel] * sc[:, None]).T
        ea17_np[c, DE, :cnt_c] = sc
        fl = np.arange(cnt_c)
        egr_np[c, fl % P, fl // P] = (egr_s[s0:s1] - c * GPC).astype(
            np.float32)
    ea17_np = ea17_np.astype(BF)
    emask_np = (ecnt.reshape(NC, GPC) > 0).astype(BF)[:, None, :]

    # ---- folded weights ----
    def fold(Wm, bm, gm, btm, mm, vm):
        s = (gm / np.sqrt(vm + EPS)).astype(np.float32)
        wf = (np.asarray(Wm, np.float32) * s[None, :]).astype(BF)
        t = (((np.asarray(bm, np.float32) - mm) * s) + btm).astype(BF)
        return wf, t[None, :]

    w1f_np, t1_np = fold(W1, b1, g1, bt1, m1, v1)
    w2f_np, t2_np = fold(W2, b2, g2, bt2, m2, v2)
    we1b_np = np.concatenate(
        [np.asarray(We1, np.float32),
         np.asarray(be1, np.float32)[None, :]], axis=0).astype(BF)

    common = {
        "w1f": w1f_np, "t1r": t1_np, "w2f": w2f_np, "t2r": t2_np,
        "we1b": we1b_np,
        "we2": np.asarray(We2, np.float32).astype(BF),
        "be2c": np.asarray(be2, np.float32)[None, :].astype(BF),
        "io128": np.tile(np.arange(P, dtype=np.float32)[None, :], (P, 1)),
        "iog": np.tile(np.arange(GPC, dtype=np.float32)[None, :], (P, 1)),
        "identd": np.eye(P, dtype=np.float32).astype(BF),
    }
    in_maps = []
    for c in range(NC):
        m = dict(common)
        m["xg"] = xg_all[c]
        m["idx2"] = idx2_np[c]
        m["meta2"] = meta2_np[c]
        m["dinvb"] = dinvb_np[c]
        m["dsqb"] = dsqb_np[c]
        m["invd"] = invd_np[c]
        m["pool_oh"] = pool_np[c]
        m["ea17"] = ea17_np[c]
        m["egr"] = egr_np[c]
        m["emask"] = emask_np[c]
        in_maps.append(m)

    sched = (tuple(K1), tuple(int(v) for v in cap2.flatten()), TE, TOT1,
             TI2, NOH2, tuple(S2G),
             tuple(tuple(s) for s in gspecs),
             tuple(tuple((w, br, fw, tuple(en)) for (w, br, fw, en) in ohs)
                   for ohs in ohspecs),
             tuple(ZW))
    return in_maps, sched


def _rv(a, m):
    return (a + m - 1) // m * m


def kernel(x, edge_index, batch_idx, edge_attr, num_graphs,
           W1, b1, g1, bt1, m1, v1, W2, b2, g2, bt2, m2, v2,
           We1, be1, We2, be2):
    in_maps, sched = _preprocess(x, edge_index, batch_idx, edge_attr,
                                 W1, b1, g1, bt1, m1, v1,
                                 W2, b2, g2, bt2, m2, v2,
                                 We1, be1, We2, be2)
    key = hash(sched)
    if _cache.get("key") != key:
        nc = _build_nc(sched)
        from concourse.bass_utils import run_bass_kernel_spmd
        _cache["key"] = key
        _cache["nc"] = nc
        _cache["run"] = lambda ims, **kw: run_bass_kernel_spmd(
            _cache["nc"], ims, list(range(NC)), **kw)
    res = _cache["run"](in_maps)
    out = np.zeros((G, D), np.float32)
    for c in range(NC):
        out[c * GPC:(c + 1) * GPC, :] = res.results[c]["out_t"].T
    return out
